# revision 38
# baseline (speedup 1.0000x reference)
"""Deformable-alignment kernel for Trainium2 (8 NeuronCores, batch-parallel).

Per core (one batch item):
  1. Pad x/ref into [128, 98*98] fp16 tiles; the padded ref is pair-expanded
     ([v,i],[v,i+1] interleaved) and stored as ONE int32 per pair so the
     bilinear gather runs with d=1 over 9604 elements.
  2. Offset/modulator conv (27 ch) as shift-im2col fp16 matmuls in PSUM;
     output transposed to pixel-major [128 pixels, 72, 27] via PE transposes.
  3. Pixel pipeline per tap on [128, 72] tiles: sampling positions, floor,
     bilinear coefs with validity masks (modulator 2x folded into deform
     weights), pair-gather base addresses -> DRAM (coefs + int16 indices).
  4. Per (1152-pixel chunk, corner row): ONE fused ap_gather covering all 9
     taps (num_idxs 10368 int32 pairs > source 9604 -> output-driven cost).
     Coefs broadcast per 3-tap group on alternating SP/Act DMA queues; DVE
     multiplies; 4-corner sum rides the PE contraction (36 accumulating
     matmuls per PSUM tile).
"""
import sys

sys.path.insert(0, "/opt/trn_rl_repo")

import numpy as np

import concourse.bass as bass
import concourse.bacc as bacc
import concourse.mybir as mybir
from concourse.tile import TileContext
from concourse.bass_utils import run_bass_kernel_spmd

B, C, H, W = 8, 128, 96, 96
HW = H * W
PH, PW = H + 2, W + 2
PHW = PH * PW
KH = KW = 3
K = KH * KW
CO = 27
NCH = 24
CHW = HW // NCH                 # 384
NF = HW // 128                  # 72 free columns in pixel-major layout
GC = 1152                       # gather chunk (pixels)
NGC = HW // GC                  # 8
SW16 = GC // 16                 # 72 wrapped-16 idx columns per (chunk, tap)
NROW = 4                        # input rows per conv tile
LGRP = 4                        # conv tiles per input-load DMA
MAGIC = float(1.5 * 2.0 ** 23)
MAXOFF = max(H, W) / 4.0

F32 = mybir.dt.float32
BF16 = mybir.dt.float16  # fp16: same speed, 8x mantissa vs bf16; ranges here are tiny
I16 = mybir.dt.int16
I32 = mybir.dt.int32
AL = mybir.AluOpType
AF = mybir.ActivationFunctionType

_CACHE = {}


def _build_program(repeat=1):
    nc = bacc.Bacc("TRN2", target_bir_lowering=False)

    x_d = nc.dram_tensor("x", [C, HW], F32, kind="ExternalInput")
    r_d = nc.dram_tensor("ref", [C, HW], F32, kind="ExternalInput")
    wconv_d = nc.dram_tensor("wconv", [2 * K * C, CO], BF16, kind="ExternalInput")
    wdef_d = nc.dram_tensor("wdef", [K * C, C], BF16, kind="ExternalInput")
    breg_d = nc.dram_tensor("breg", [C, 1], F32, kind="ExternalInput")
    bmod_d = nc.dram_tensor("bmod", [128, K, NF], F32, kind="ExternalInput")
    hkmap_d = nc.dram_tensor("hkmap", [128, K, NF], F32, kind="ExternalInput")
    wkmap_d = nc.dram_tensor("wkmap", [128, K, NF], F32, kind="ExternalInput")
    id27_d = nc.dram_tensor("id27", [CO, CO], F32, kind="ExternalInput")
    y_d = nc.dram_tensor("y", [C, HW], F32, kind="ExternalOutput")

    with TileContext(nc) as tc:
        with (
            tc.tile_pool(name="const", bufs=1) as cpool,
            tc.tile_pool(name="inp", bufs=1) as ipool,
            tc.tile_pool(name="wix", bufs=1) as wpool,
            tc.tile_pool(name="dsc", bufs=1, space="DRAM") as dpool,
        ):
            # ---------- long-lived constants ----------
            wdef_sb = cpool.tile([128, K, C], BF16)
            nc.sync.dma_start(wdef_sb[:], wdef_d[:].rearrange("(a p) o -> p a o", p=128))
            breg_sb = cpool.tile([C, 1], F32)
            nc.sync.dma_start(breg_sb[:], breg_d[:])

            # padded ref, one int32 per (v[i], v[i+1]) fp16 pair
            rpair32 = ipool.tile([C, PHW], I32, tag="rpair32")
            # wrapped-16 gather indices per corner row: [128, chunk, tap, GC/16]
            widx_ys = [wpool.tile([128, NGC, K, SW16], I16, tag=f"widx{ys}",
                                  name=f"widx{ys}")
                       for ys in range(2)]

            # per-ys rows in (tap, pixel)-major interleaved order: one store each
            cp_dram = dpool.tile([1, 2, K, HW * 2], BF16, tag="cpd")
            idx_dram = dpool.tile([2, K * HW], I16, tag="idxd")

            for _rep in range(repeat):
              with (
                tc.tile_pool(name=f"pix{_rep}", bufs=1) as xpool,
                tc.tile_pool(name=f"cb{_rep}", bufs=2) as bpool,
                tc.tile_pool(name=f"sg{_rep}", bufs=1) as sgpool,
                tc.tile_pool(name="pk", bufs=1) as kpool,
                tc.tile_pool(name="psc", bufs=2, space="PSUM") as pconv,
                tc.tile_pool(name="pst", bufs=2, space="PSUM") as ptr,
              ):
                # ---------- stage-local constants ----------
                wconv_sb = xpool.tile([128, 2 * K, CO], BF16, tag="wconv")
                nc.sync.dma_start(wconv_sb[:], wconv_d[:].rearrange("(a p) o -> p a o", p=128))
                hkmap_sb = xpool.tile([128, K, NF], F32, tag="hkmap")
                nc.sync.dma_start(hkmap_sb[:], hkmap_d[:])
                wkmap_sb = xpool.tile([128, K, NF], F32, tag="wkmap")
                nc.sync.dma_start(wkmap_sb[:], wkmap_d[:])
                id27_sb = xpool.tile([CO, CO], F32, tag="id27")
                nc.sync.dma_start(id27_sb[:], id27_d[:])
                bmk_sb = xpool.tile([128, K, NF], F32, tag="bmk")
                nc.sync.dma_start(bmk_sb[:], bmod_d[:])

                # ---------- stage A: padded fp16 inputs ----------
                xc0 = xpool.tile([C, PHW], BF16, tag="xc0")
                xg = xc0[:].rearrange("p (h w) -> p h w", h=PH)
                nc.vector.memset(xg[:, 0, :], 0.0)
                nc.vector.memset(xg[:, PH - 1, :], 0.0)
                nc.vector.memset(xg[:, 1 : PH - 1, 0], 0.0)
                nc.vector.memset(xg[:, 1 : PH - 1, PW - 1], 0.0)

                rvb = rpair32[:].bitcast(BF16)                      # [C, 2*PHW]
                rv3 = rvb.rearrange("p (h w j) -> p h w j", h=PH, j=2)
                nc.vector.memset(rv3[:, 0, :, 0], 0.0)
                nc.vector.memset(rv3[:, PH - 1, :, 0], 0.0)
                nc.vector.memset(rv3[:, 1 : PH - 1, 0, 0], 0.0)
                nc.vector.memset(rv3[:, 1 : PH - 1, PW - 1, 0], 0.0)

                LCH = LGRP * CHW            # pixels per load DMA
                LR = LGRP * NROW            # rows per load DMA
                for n in range(NCH // LGRP):
                    cb = bpool.tile([C, LCH], F32, tag="cbuf", name="cb")
                    nc.sync.dma_start(cb[:], x_d[:, n * LCH : (n + 1) * LCH])
                    nc.vector.tensor_copy(
                        xg[:, 1 + LR * n : 1 + LR * (n + 1), 1 : 1 + W],
                        cb[:].rearrange("p (h w) -> p h w", h=LR))
                for n in range(NCH // LGRP):
                    cb2 = bpool.tile([C, LCH], F32, tag="cbuf2", name="cb2")
                    nc.scalar.dma_start(cb2[:], r_d[:, n * LCH : (n + 1) * LCH])
                    nc.vector.tensor_copy(
                        rv3[:, 1 + LR * n : 1 + LR * (n + 1), 1 : 1 + W, 0],
                        cb2[:].rearrange("p (h w) -> p h w", h=LR))
                rp2 = rvb.rearrange("p (a j) -> p a j", j=2)
                nc.scalar.activation(rp2[:, 0 : PHW - 1, 1], rp2[:, 1:PHW, 0], AF.Copy)
                nc.vector.memset(rp2[:, PHW - 1 : PHW, 1], 0.0)

                # ---------- stage B: conv + transpose to pixel-major ----------
                # PPIX channel-outer: rows 0-8 = y-offsets, 9-17 = x-offsets,
                # 18-26 = modulator (host permutes wconv columns accordingly)
                PPIX = xpool.tile([128, CO, NF], F32, tag="PPIX")
                xv0 = xg
                xv1 = rv3[:, :, :, 0]
                for n in range(NCH):
                    ps = pconv.tile([CO, CHW], F32, tag="convps", name="ps")
                    h0 = n * NROW
                    mi = 0
                    for xv in (xv0, xv1):
                        for ky in range(KH):
                            for kx in range(KW):
                                rhs = xv[:, h0 + ky : h0 + ky + NROW, kx : kx + W]
                                cb_i = 0 if xv is xv0 else 1
                                nc.tensor.matmul(
                                    ps[:], wconv_sb[:, cb_i * K + ky * KW + kx, :], rhs,
                                    start=(mi == 0), stop=(mi == 17))
                                mi += 1
                    t27 = xpool.tile([CO, CHW], F32, tag="t27", name="t27")
                    nc.scalar.activation(t27[:], ps[:], AF.Copy)
                    for s in range(3):
                        pt = ptr.tile([128, CO], F32, tag="trps", name="pt")
                        nc.tensor.transpose(pt[:], t27[:, s * 128 : (s + 1) * 128], id27_sb[:])
                        nc.vector.tensor_copy(PPIX[:, :, n * 3 + s], pt[:])

                # ---------- stage C: pixel pipeline, all taps stacked [128, 9*72] ----------
                KF = K * NF

                def ts1(out, in_, s, op):
                    nc.vector.tensor_scalar(out=out, in0=in_, scalar1=float(s), scalar2=None, op0=op)

                def ts2(out, in_, s1, s2, op0=AL.max, op1=AL.min):
                    nc.vector.tensor_scalar(
                        out=out, in0=in_, scalar1=float(s1), scalar2=float(s2), op0=op0, op1=op1)

                def kt(tag):
                    return kpool.tile([128, KF], F32, tag=tag, name=tag)

                # --- phase 1: sampling positions + floor only ---
                pos = {}
                for side, mp, ch0 in (("y", hkmap_sb, 0), ("x", wkmap_sb, K)):
                    pview = PPIX[:, ch0 : ch0 + K, :].rearrange("p k f -> p (k f)")
                    p_ = kt(f"p{side}")
                    nc.vector.tensor_tensor(p_[:], pview, mp[:].rearrange("p k f -> p (k f)"), op=AL.add)
                    z0 = kt(f"z0{side}")
                    ts2(z0[:], p_[:], MAGIC, MAGIC, AL.add, AL.subtract)
                    z1 = kt(f"z1{side}")
                    ts1(z1[:], z0[:], 1.0, AL.add)
                    pos[side] = (p_, z0, z1)
                y1 = pos["y"][2]
                x1 = pos["x"][2]
                # --- index branch: unblocks the widx chain early ---
                xb = kt("xb")
                ts2(xb[:], x1[:], 0.0, float(PW - 1))
                r0 = kt("r0")
                ts2(r0[:], y1[:], 0.0, float(PH - 1))
                r1 = kt("r1")
                ts1(r1[:], y1[:], 1.0, AL.add)
                ts2(r1[:], r1[:], 0.0, float(PH - 1))
                for ys, rr in ((0, r0), (1, r1)):
                    if_ = kt("scr")
                    nc.vector.scalar_tensor_tensor(
                        out=if_[:], in0=rr[:], scalar=float(PW), in1=xb[:],
                        op0=AL.mult, op1=AL.add)
                    ii = kpool.tile([128, KF], I16, tag=f"ii{ys}", name="ii")
                    nc.vector.tensor_copy(ii[:], if_[:])
                    eng0 = nc.sync if ys == 0 else nc.scalar
                    eng1 = nc.scalar if ys == 0 else nc.sync
                    wt = widx_ys[ys]
                    with tc.high_priority():
                        dsti = idx_dram[ys, :].rearrange("(kf p) -> p kf", p=128)
                        (nc.sync if ys == 0 else nc.scalar).dma_start(dsti, ii[:])
                        # widx chain for this ys: staged load (split), 9 scatters, replicate
                        stg = sgpool.tile([16, K * HW // 16], I16, tag=f"stg{ys}", name="stg")
                        src = idx_dram[ys, :].rearrange("(s p) -> p s", p=16)
                        HSG = K * HW // 32
                        eng1.dma_start(stg[:, 0:HSG], src[:, 0:HSG])
                        eng0.dma_start(stg[:, HSG:], src[:, HSG:])
                        for k in range(K):
                            eng0.dma_start(
                                wt[0:16, :, k, :],
                                stg[:, k * (HW // 16) : (k + 1) * (HW // 16)]
                                .rearrange("p (c us) -> p c us", us=SW16))
                        ranges = ((slice(0, 2), slice(2, NGC)) if ys == 0
                                  else (slice(0, NGC),))
                        for cs in ranges:
                            p = 16
                            while p < 128:
                                eng0.dma_start(wt[p : 2 * p, cs], wt[0:p, cs])
                                p *= 2

                # --- phase 2: bilinear weights + validity ---
                res = {}
                for side in ("y", "x"):
                    p_, z0, z1 = pos[side]
                    wf = kt(f"wf{side}")
                    nc.vector.tensor_tensor(wf[:], p_[:], z0[:], op=AL.subtract)
                    cl = kt("scr")
                    ts2(cl[:], z0[:], 0.0, float(H - 1))
                    v0 = kt(f"v0{side}")
                    nc.vector.tensor_tensor(v0[:], z0[:], cl[:], op=AL.is_equal)
                    cl1 = kt("scr")
                    ts2(cl1[:], z1[:], 0.0, float(H - 1))
                    v1 = kt(f"v1{side}")
                    nc.vector.tensor_tensor(v1[:], z1[:], cl1[:], op=AL.is_equal)
                    a0 = kt(f"a0{side}")
                    ts2(a0[:], wf[:], -1.0, 0.5, AL.mult, AL.add)
                    nc.vector.tensor_tensor(a0[:], a0[:], v0[:], op=AL.mult)
                    a1 = kt(f"a1{side}")
                    nc.vector.scalar_tensor_tensor(
                        out=a1[:], in0=wf[:], scalar=0.5, in1=v1[:],
                        op0=AL.add, op1=AL.mult)
                    res[side] = (a0, a1)
                a0y, a1y = res["y"]
                a0x, a1x = res["x"]
                # --- coefficient branch ---
                msin = kt("scr")
                nc.vector.tensor_tensor(
                    msin[:], PPIX[:, 2 * K : 3 * K, :].rearrange("p k f -> p (k f)"),
                    bmk_sb[:].rearrange("p k f -> p (k f)"), op=AL.add)
                ms = kt("ms")
                nc.scalar.activation(ms[:], msin[:], AF.Sigmoid)
                ty0 = kt("ty0")
                nc.vector.tensor_tensor(ty0[:], ms[:], a0y[:], op=AL.mult)
                ty1 = kt("ty1")
                nc.vector.tensor_tensor(ty1[:], ms[:], a1y[:], op=AL.mult)
                cp0 = kpool.tile([128, KF, 2], BF16, tag="cp0", name="cp0")
                cp1 = kpool.tile([128, KF, 2], BF16, tag="cp1", name="cp1")
                nc.vector.tensor_tensor(cp0[:, :, 0], ty0[:], a0x[:], op=AL.mult)
                nc.vector.tensor_tensor(cp0[:, :, 1], ty0[:], a1x[:], op=AL.mult)
                nc.vector.tensor_tensor(cp1[:, :, 0], ty1[:], a0x[:], op=AL.mult)
                nc.vector.tensor_tensor(cp1[:, :, 1], ty1[:], a1x[:], op=AL.mult)
                for ys, cp in ((0, cp0), (1, cp1)):
                    eng = nc.sync if ys == 0 else nc.scalar
                    dst = cp_dram[0, ys].rearrange("k (f p j) -> p (k f) j", p=128, j=2)
                    eng.dma_start(dst, cp[:])

              # ---------- stages E+F: fused gather, coef multiply, matmul ----------
              with (
                tc.tile_pool(name=f"gat{_rep}", bufs=1) as gpool,
                tc.tile_pool(name=f"crp{_rep}", bufs=2) as rpool,
                tc.tile_pool(name=f"mm{_rep}", bufs=2) as mpool,
                tc.tile_pool(name=f"out{_rep}", bufs=2) as opool,
                tc.tile_pool(name=f"psd{_rep}", bufs=2, space="PSUM") as pdef,
              ):
                NSUB = GC // CHW        # 3
                dmai = 0
                for c in range(NGC):
                    pss = []
                    for s in range(NSUB):
                        dtile = pdef.tile([C, CHW], F32, tag=f"dps{s}", name="dtile")
                        pss.append(dtile)
                    for ys in range(2):
                        g = gpool.tile([C, K * GC], I32, tag=f"g{ys}", name=f"g{ys}")
                        nc.gpsimd.ap_gather(
                            g[:], rpair32[:], widx_ys[ys][:, c],
                            channels=128, num_elems=PHW, d=1, num_idxs=K * GC)
                        for gi in range(3):
                            k0 = gi * 3
                            crg = rpool.tile([C, 3, GC * 2], BF16, tag="crg", name="crg")
                            eng = nc.sync if dmai % 2 == 0 else nc.scalar
                            dmai += 1
                            eng.dma_start(
                                crg[:], cp_dram[:, ys, k0 : k0 + 3, c * GC * 2 : (c + 1) * GC * 2]
                                .to_broadcast((C, 3, GC * 2)))
                            m3 = mpool.tile([C, 3 * GC, 2], BF16, tag="m", name="m3")
                            gk = g[:, k0 * GC : (k0 + 3) * GC].bitcast(BF16)
                            nc.vector.tensor_tensor(
                                m3[:].rearrange("p a b -> p (a b)"), gk,
                                crg[:].rearrange("p a b -> p (a b)"), op=AL.mult)
                            for ki in range(3):
                                k = k0 + ki
                                for s in range(NSUB):
                                    sl = slice(ki * GC + s * CHW, ki * GC + (s + 1) * CHW)
                                    for lane in (0, 1):
                                        nc.tensor.matmul(
                                            pss[s][:], wdef_sb[:, k, :], m3[:, sl, lane],
                                            start=(ys == 0 and k == 0 and lane == 0),
                                            stop=(ys == 1 and k == K - 1 and lane == 1))
                    for s in range(NSUB):
                        ot = opool.tile([C, CHW], F32, tag="out", name="ot")
                        nc.scalar.activation(ot[:], pss[s][:], AF.Identity, bias=breg_sb[:])
                        q0 = c * GC + s * CHW
                        nc.sync.dma_start(y_d[:, q0 : q0 + CHW], ot[:])

    nc.finalize()
    return nc


def _host_maps(b_off):
    q = np.arange(HW)
    p, f = q % 128, q // 128
    hh, ww = (q // W).astype(np.float32), (q % W).astype(np.float32)
    hk = np.zeros((128, K, NF), np.float32)
    wk = np.zeros((128, K, NF), np.float32)
    for k in range(K):
        ky, kx = k // KW, k % KW
        hk[p, k, f] = hh + (ky - 1) + np.float32(b_off[2 * k]) - 0.5
        wk[p, k, f] = ww + (kx - 1) + np.float32(b_off[2 * k + 1]) - 0.5
    return hk, wk


def kernel(x, ref_feats, w_off, b_off, w_mod, b_mod, w_reg, b_reg):
    if "nc" not in _CACHE:
        _CACHE["nc"] = _build_program()
    nc = _CACHE["nc"]

    w_all = np.concatenate([w_off, w_mod], axis=0).astype(np.float32)
    wc = w_all.reshape(CO, 2, 128, KH, KW).transpose(1, 3, 4, 2, 0)
    wconv = np.ascontiguousarray(wc.reshape(2 * K * C, CO))
    # channel-outer PPIX order: [y-offsets, x-offsets, modulator]
    perm = [2 * k for k in range(K)] + [2 * k + 1 for k in range(K)] + list(range(2 * K, CO))
    wconv = np.ascontiguousarray(wconv[:, perm])
    # modulator = 2*sigmoid -> fold the 2x into the deform weights
    wd = (2.0 * np.asarray(w_reg, np.float32)).reshape(C, C, K).transpose(2, 1, 0)
    wdef = np.ascontiguousarray(wd.reshape(K * C, C))
    hk, wk = _host_maps(np.asarray(b_off, np.float32))

    bmk = np.broadcast_to(
        np.asarray(b_mod, np.float32)[None, :, None], (128, K, NF))
    shared = dict(
        wconv=wconv.astype(np.float16), wdef=wdef.astype(np.float16),
        breg=np.asarray(b_reg, np.float32)[:, None],
        bmod=np.ascontiguousarray(bmk),
        hkmap=hk, wkmap=wk, id27=np.eye(CO, dtype=np.float32),
    )
    in_maps = []
    for b in range(B):
        m = dict(shared)
        m["x"] = np.ascontiguousarray(np.asarray(x[b], np.float32).reshape(C, HW))
        m["ref"] = np.ascontiguousarray(np.asarray(ref_feats[b], np.float32).reshape(C, HW))
        in_maps.append(m)
    _CACHE["in_maps"] = in_maps

    res = run_bass_kernel_spmd(nc, in_maps, core_ids=list(range(B)))
    out = np.stack([np.asarray(res.results[b]["y"]).reshape(C, H, W) for b in range(B)])
    return out.astype(np.float32)


# revision 39
# speedup vs baseline: 1.2092x; 1.2092x over previous
"""Deformable-alignment kernel for Trainium2 (8 NeuronCores, batch-parallel).

Per core (one batch item):
  1. Pad x/ref into [128, 98*98] fp16 tiles; the padded ref is pair-expanded
     ([v,i],[v,i+1] interleaved) and stored as ONE int32 per pair so the
     bilinear gather runs with d=1 over 9604 elements.
  2. Offset/modulator conv (27 ch) as shift-im2col fp16 matmuls in PSUM;
     output transposed to pixel-major [128 pixels, 72, 27] via PE transposes.
  3. Pixel pipeline per tap on [128, 72] tiles: sampling positions, floor,
     bilinear coefs with validity masks (modulator 2x folded into deform
     weights), pair-gather base addresses -> DRAM (coefs + int16 indices).
  4. Per (1152-pixel chunk, corner row): ONE fused ap_gather covering all 9
     taps (num_idxs 10368 int32 pairs > source 9604 -> output-driven cost).
     Coefs broadcast per 3-tap group on alternating SP/Act DMA queues; DVE
     multiplies; 4-corner sum rides the PE contraction (36 accumulating
     matmuls per PSUM tile).
"""
import sys

sys.path.insert(0, "/opt/trn_rl_repo")

import numpy as np

import concourse.bass as bass
import concourse.bacc as bacc
import concourse.mybir as mybir
from concourse.tile import TileContext
from concourse.bass_utils import run_bass_kernel_spmd

B, C, H, W = 8, 128, 96, 96
HW = H * W
PH, PW = H + 2, W + 2
PHW = PH * PW
KH = KW = 3
K = KH * KW
CO = 27
NCH = 24
CHW = HW // NCH                 # 384
NF = HW // 128                  # 72 free columns in pixel-major layout
GC = 1152                       # gather chunk (pixels)
NGC = HW // GC                  # 8
SW16 = GC // 16                 # 72 wrapped-16 idx columns per (chunk, tap)
NROW = 4                        # input rows per conv tile
LGRP = 4                        # conv tiles per input-load DMA
MAGIC = float(1.5 * 2.0 ** 23)
MAXOFF = max(H, W) / 4.0

F32 = mybir.dt.float32
BF16 = mybir.dt.float16  # fp16: same speed, 8x mantissa vs bf16; ranges here are tiny
I16 = mybir.dt.int16
I32 = mybir.dt.int32
AL = mybir.AluOpType
AF = mybir.ActivationFunctionType

_CACHE = {}


def _build_program(repeat=1):
    nc = bacc.Bacc("TRN2", target_bir_lowering=False)

    x_d = nc.dram_tensor("x", [C, HW], F32, kind="ExternalInput")
    r_d = nc.dram_tensor("ref", [C, HW], F32, kind="ExternalInput")
    wconv_d = nc.dram_tensor("wconv", [2 * K * C, CO], BF16, kind="ExternalInput")
    wdef_d = nc.dram_tensor("wdef", [K * C, C], BF16, kind="ExternalInput")
    breg_d = nc.dram_tensor("breg", [C, 1], F32, kind="ExternalInput")
    bmod_d = nc.dram_tensor("bmod", [128, K, NF], F32, kind="ExternalInput")
    hkmap_d = nc.dram_tensor("hkmap", [128, K, NF], F32, kind="ExternalInput")
    wkmap_d = nc.dram_tensor("wkmap", [128, K, NF], F32, kind="ExternalInput")
    id27_d = nc.dram_tensor("id27", [CO, CO], F32, kind="ExternalInput")
    y_d = nc.dram_tensor("y", [C, HW], F32, kind="ExternalOutput")

    with TileContext(nc) as tc:
        with (
            tc.tile_pool(name="const", bufs=1) as cpool,
            tc.tile_pool(name="inp", bufs=1) as ipool,
            tc.tile_pool(name="wix", bufs=1) as wpool,
            tc.tile_pool(name="dsc", bufs=1, space="DRAM") as dpool,
        ):
            # ---------- long-lived constants ----------
            wdef_sb = cpool.tile([128, K, C], BF16)
            nc.sync.dma_start(wdef_sb[:], wdef_d[:].rearrange("(a p) o -> p a o", p=128))
            breg_sb = cpool.tile([C, 1], F32)
            nc.sync.dma_start(breg_sb[:], breg_d[:])

            # padded ref, one int32 per (v[i], v[i+1]) fp16 pair
            rpair32 = ipool.tile([C, PHW], I32, tag="rpair32")
            # wrapped-16 gather indices per corner row: [128, chunk, tap, GC/16]
            widx_ys = [wpool.tile([128, NGC, K, SW16], I16, tag=f"widx{ys}",
                                  name=f"widx{ys}")
                       for ys in range(2)]

            # per-ys rows in (tap, pixel)-major interleaved order: one store each
            cp_dram = dpool.tile([1, 2, K, HW * 2], BF16, tag="cpd")
            idx_dram = dpool.tile([2, K * HW], I16, tag="idxd")

            for _rep in range(repeat):
              with (
                tc.tile_pool(name=f"pix{_rep}", bufs=1) as xpool,
                tc.tile_pool(name=f"cb{_rep}", bufs=2) as bpool,
                tc.tile_pool(name=f"sg{_rep}", bufs=1) as sgpool,
                tc.tile_pool(name="pk", bufs=1) as kpool,
                tc.tile_pool(name="psc", bufs=2, space="PSUM") as pconv,
                tc.tile_pool(name="pst", bufs=2, space="PSUM") as ptr,
              ):
                # ---------- stage-local constants ----------
                wconv_sb = xpool.tile([128, 2 * K, CO], BF16, tag="wconv")
                nc.sync.dma_start(wconv_sb[:], wconv_d[:].rearrange("(a p) o -> p a o", p=128))
                hkmap_sb = xpool.tile([128, K, NF], F32, tag="hkmap")
                nc.sync.dma_start(hkmap_sb[:], hkmap_d[:])
                wkmap_sb = xpool.tile([128, K, NF], F32, tag="wkmap")
                nc.sync.dma_start(wkmap_sb[:], wkmap_d[:])
                id27_sb = xpool.tile([CO, CO], F32, tag="id27")
                nc.sync.dma_start(id27_sb[:], id27_d[:])
                bmk_sb = xpool.tile([128, K, NF], F32, tag="bmk")
                nc.sync.dma_start(bmk_sb[:], bmod_d[:])

                # ---------- stage A: padded fp16 inputs ----------
                xc0 = xpool.tile([C, PHW], BF16, tag="xc0")
                xg = xc0[:].rearrange("p (h w) -> p h w", h=PH)
                nc.vector.memset(xg[:, 0, :], 0.0)
                nc.vector.memset(xg[:, PH - 1, :], 0.0)
                nc.vector.memset(xg[:, 1 : PH - 1, 0], 0.0)
                nc.vector.memset(xg[:, 1 : PH - 1, PW - 1], 0.0)

                rvb = rpair32[:].bitcast(BF16)                      # [C, 2*PHW]
                rv3 = rvb.rearrange("p (h w j) -> p h w j", h=PH, j=2)
                nc.vector.memset(rv3[:, 0, :, 0], 0.0)
                nc.vector.memset(rv3[:, PH - 1, :, 0], 0.0)
                nc.vector.memset(rv3[:, 1 : PH - 1, 0, 0], 0.0)
                nc.vector.memset(rv3[:, 1 : PH - 1, PW - 1, 0], 0.0)

                LCH = LGRP * CHW            # pixels per load DMA
                LR = LGRP * NROW            # rows per load DMA
                for n in range(NCH // LGRP):
                    cb = bpool.tile([C, LCH], F32, tag="cbuf", name="cb")
                    nc.sync.dma_start(cb[:], x_d[:, n * LCH : (n + 1) * LCH])
                    nc.vector.tensor_copy(
                        xg[:, 1 + LR * n : 1 + LR * (n + 1), 1 : 1 + W],
                        cb[:].rearrange("p (h w) -> p h w", h=LR))
                for n in range(NCH // LGRP):
                    cb2 = bpool.tile([C, LCH], F32, tag="cbuf2", name="cb2")
                    nc.scalar.dma_start(cb2[:], r_d[:, n * LCH : (n + 1) * LCH])
                    nc.vector.tensor_copy(
                        rv3[:, 1 + LR * n : 1 + LR * (n + 1), 1 : 1 + W, 0],
                        cb2[:].rearrange("p (h w) -> p h w", h=LR))
                rp2 = rvb.rearrange("p (a j) -> p a j", j=2)
                nc.scalar.activation(rp2[:, 0 : PHW - 1, 1], rp2[:, 1:PHW, 0], AF.Copy)
                nc.vector.memset(rp2[:, PHW - 1 : PHW, 1], 0.0)

                # ---------- stage B: conv + transpose to pixel-major ----------
                # PPIX channel-outer: rows 0-8 = y-offsets, 9-17 = x-offsets,
                # 18-26 = modulator (host permutes wconv columns accordingly)
                PPIX = xpool.tile([128, CO, NF], F32, tag="PPIX")
                xv0 = xg
                xv1 = rv3[:, :, :, 0]
                for n in range(NCH):
                    ps = pconv.tile([CO, CHW], F32, tag="convps", name="ps")
                    h0 = n * NROW
                    mi = 0
                    for xv in (xv0, xv1):
                        for ky in range(KH):
                            for kx in range(KW):
                                rhs = xv[:, h0 + ky : h0 + ky + NROW, kx : kx + W]
                                cb_i = 0 if xv is xv0 else 1
                                nc.tensor.matmul(
                                    ps[:], wconv_sb[:, cb_i * K + ky * KW + kx, :], rhs,
                                    start=(mi == 0), stop=(mi == 17))
                                mi += 1
                    t27 = xpool.tile([CO, CHW], F32, tag="t27", name="t27")
                    nc.scalar.activation(t27[:], ps[:], AF.Copy)
                    for s in range(3):
                        pt = ptr.tile([128, CO], F32, tag="trps", name="pt")
                        nc.tensor.transpose(pt[:], t27[:, s * 128 : (s + 1) * 128], id27_sb[:])
                        nc.vector.tensor_copy(PPIX[:, :, n * 3 + s], pt[:])

                # ---------- stage C: pixel pipeline, all taps stacked [128, 9*72] ----------
                KF = K * NF

                def ts1(out, in_, s, op):
                    nc.vector.tensor_scalar(out=out, in0=in_, scalar1=float(s), scalar2=None, op0=op)

                def ts2(out, in_, s1, s2, op0=AL.max, op1=AL.min):
                    nc.vector.tensor_scalar(
                        out=out, in0=in_, scalar1=float(s1), scalar2=float(s2), op0=op0, op1=op1)

                def kt(tag):
                    return kpool.tile([128, KF], F32, tag=tag, name=tag)

                # --- phase 1: sampling positions + floor only ---
                pos = {}
                for side, mp, ch0 in (("y", hkmap_sb, 0), ("x", wkmap_sb, K)):
                    pview = PPIX[:, ch0 : ch0 + K, :].rearrange("p k f -> p (k f)")
                    p_ = kt(f"p{side}")
                    nc.vector.tensor_tensor(p_[:], pview, mp[:].rearrange("p k f -> p (k f)"), op=AL.add)
                    z0 = kt(f"z0{side}")
                    ts2(z0[:], p_[:], MAGIC, MAGIC, AL.add, AL.subtract)
                    z1 = kt(f"z1{side}")
                    ts1(z1[:], z0[:], 1.0, AL.add)
                    pos[side] = (p_, z0, z1)
                y1 = pos["y"][2]
                x1 = pos["x"][2]
                # --- index branch: unblocks the widx chain early ---
                xb = kt("xb")
                ts2(xb[:], x1[:], 0.0, float(PW - 1))
                r0 = kt("r0")
                ts2(r0[:], y1[:], 0.0, float(PH - 1))
                r1 = kt("r1")
                ts1(r1[:], y1[:], 1.0, AL.add)
                ts2(r1[:], r1[:], 0.0, float(PH - 1))
                for ys, rr in ((0, r0), (1, r1)):
                    if_ = kt("scr")
                    nc.vector.scalar_tensor_tensor(
                        out=if_[:], in0=rr[:], scalar=float(PW), in1=xb[:],
                        op0=AL.mult, op1=AL.add)
                    ii = kpool.tile([128, KF], I16, tag=f"ii{ys}", name="ii")
                    nc.vector.tensor_copy(ii[:], if_[:])
                    eng0 = nc.sync if ys == 0 else nc.scalar
                    eng1 = nc.scalar if ys == 0 else nc.sync
                    wt = widx_ys[ys]
                    with tc.high_priority():
                        dsti = idx_dram[ys, :].rearrange("(kf p) -> p kf", p=128)
                        (nc.sync if ys == 0 else nc.scalar).dma_start(dsti, ii[:])
                        # widx chain for this ys: one staging load, 9 scatters, replicate
                        stg = sgpool.tile([16, K * HW // 16], I16, tag=f"stg{ys}", name="stg")
                        src = idx_dram[ys, :].rearrange("(s p) -> p s", p=16)
                        eng1.dma_start(stg[:], src)
                        for k in range(K):
                            eng0.dma_start(
                                wt[0:16, :, k, :],
                                stg[:, k * (HW // 16) : (k + 1) * (HW // 16)]
                                .rearrange("p (c us) -> p c us", us=SW16))
                        ranges = ((slice(0, 2), slice(2, NGC)) if ys == 0
                                  else (slice(0, NGC),))
                        for cs in ranges:
                            p = 16
                            while p < 128:
                                eng0.dma_start(wt[p : 2 * p, cs], wt[0:p, cs])
                                p *= 2

                # --- phase 2: bilinear weights + validity ---
                res = {}
                for side in ("y", "x"):
                    p_, z0, z1 = pos[side]
                    wf = kt(f"wf{side}")
                    nc.vector.tensor_tensor(wf[:], p_[:], z0[:], op=AL.subtract)
                    cl = kt("scr")
                    ts2(cl[:], z0[:], 0.0, float(H - 1))
                    v0 = kt(f"v0{side}")
                    nc.vector.tensor_tensor(v0[:], z0[:], cl[:], op=AL.is_equal)
                    cl1 = kt("scr")
                    ts2(cl1[:], z1[:], 0.0, float(H - 1))
                    v1 = kt(f"v1{side}")
                    nc.vector.tensor_tensor(v1[:], z1[:], cl1[:], op=AL.is_equal)
                    a0 = kt(f"a0{side}")
                    ts2(a0[:], wf[:], -1.0, 0.5, AL.mult, AL.add)
                    nc.vector.tensor_tensor(a0[:], a0[:], v0[:], op=AL.mult)
                    a1 = kt(f"a1{side}")
                    nc.vector.scalar_tensor_tensor(
                        out=a1[:], in0=wf[:], scalar=0.5, in1=v1[:],
                        op0=AL.add, op1=AL.mult)
                    res[side] = (a0, a1)
                a0y, a1y = res["y"]
                a0x, a1x = res["x"]
                # --- coefficient branch ---
                msin = kt("scr")
                nc.vector.tensor_tensor(
                    msin[:], PPIX[:, 2 * K : 3 * K, :].rearrange("p k f -> p (k f)"),
                    bmk_sb[:].rearrange("p k f -> p (k f)"), op=AL.add)
                ms = kt("ms")
                nc.scalar.activation(ms[:], msin[:], AF.Sigmoid)
                ty0 = kt("ty0")
                nc.vector.tensor_tensor(ty0[:], ms[:], a0y[:], op=AL.mult)
                ty1 = kt("ty1")
                nc.vector.tensor_tensor(ty1[:], ms[:], a1y[:], op=AL.mult)
                cp0 = kpool.tile([128, KF, 2], BF16, tag="cp0", name="cp0")
                cp1 = kpool.tile([128, KF, 2], BF16, tag="cp1", name="cp1")
                nc.vector.tensor_tensor(cp0[:, :, 0], ty0[:], a0x[:], op=AL.mult)
                nc.vector.tensor_tensor(cp0[:, :, 1], ty0[:], a1x[:], op=AL.mult)
                nc.vector.tensor_tensor(cp1[:, :, 0], ty1[:], a0x[:], op=AL.mult)
                nc.vector.tensor_tensor(cp1[:, :, 1], ty1[:], a1x[:], op=AL.mult)
                for ys, cp in ((0, cp0), (1, cp1)):
                    eng = nc.sync if ys == 0 else nc.scalar
                    dst = cp_dram[0, ys].rearrange("k (f p j) -> p (k f) j", p=128, j=2)
                    eng.dma_start(dst, cp[:])

              # ---------- stages E+F: fused gather, coef multiply, matmul ----------
              with (
                tc.tile_pool(name=f"gat{_rep}", bufs=1) as gpool,
                tc.tile_pool(name=f"crp{_rep}", bufs=3) as rpool,
                tc.tile_pool(name=f"mm{_rep}", bufs=3) as mpool,
                tc.tile_pool(name=f"out{_rep}", bufs=2) as opool,
                tc.tile_pool(name=f"psd{_rep}", bufs=2, space="PSUM") as pdef,
              ):
                NSUB = GC // CHW        # 3
                dmai = 0
                for c in range(NGC):
                    pss = []
                    for s in range(NSUB):
                        dtile = pdef.tile([C, CHW], F32, tag=f"dps{s}", name="dtile")
                        pss.append(dtile)
                    for ys in range(2):
                        g = gpool.tile([C, K * GC], I32, tag=f"g{ys}", name=f"g{ys}")
                        nc.gpsimd.ap_gather(
                            g[:], rpair32[:], widx_ys[ys][:, c],
                            channels=128, num_elems=PHW, d=1, num_idxs=K * GC)
                        for gi in range(3):
                            k0 = gi * 3
                            crg = rpool.tile([C, 3, GC * 2], BF16, tag="crg", name="crg")
                            eng = nc.sync if dmai % 2 == 0 else nc.scalar
                            dmai += 1
                            eng.dma_start(
                                crg[:], cp_dram[:, ys, k0 : k0 + 3, c * GC * 2 : (c + 1) * GC * 2]
                                .to_broadcast((C, 3, GC * 2)))
                            for ki in range(3):
                                k = k0 + ki
                                m = mpool.tile([C, GC, 2], BF16, tag="m", name="m")
                                gk = g[:, k * GC : (k + 1) * GC].bitcast(BF16)
                                nc.vector.tensor_tensor(
                                    m[:].rearrange("p a b -> p (a b)"), gk, crg[:, ki, :],
                                    op=AL.mult)
                                for s in range(NSUB):
                                    sl = slice(s * CHW, (s + 1) * CHW)
                                    for lane in (0, 1):
                                        nc.tensor.matmul(
                                            pss[s][:], wdef_sb[:, k, :], m[:, sl, lane],
                                            start=(ys == 0 and k == 0 and lane == 0),
                                            stop=(ys == 1 and k == K - 1 and lane == 1))
                    for s in range(NSUB):
                        ot = opool.tile([C, CHW], F32, tag="out", name="ot")
                        nc.scalar.activation(ot[:], pss[s][:], AF.Identity, bias=breg_sb[:])
                        q0 = c * GC + s * CHW
                        nc.sync.dma_start(y_d[:, q0 : q0 + CHW], ot[:])

    nc.finalize()
    return nc


def _host_maps(b_off):
    q = np.arange(HW)
    p, f = q % 128, q // 128
    hh, ww = (q // W).astype(np.float32), (q % W).astype(np.float32)
    hk = np.zeros((128, K, NF), np.float32)
    wk = np.zeros((128, K, NF), np.float32)
    for k in range(K):
        ky, kx = k // KW, k % KW
        hk[p, k, f] = hh + (ky - 1) + np.float32(b_off[2 * k]) - 0.5
        wk[p, k, f] = ww + (kx - 1) + np.float32(b_off[2 * k + 1]) - 0.5
    return hk, wk


def kernel(x, ref_feats, w_off, b_off, w_mod, b_mod, w_reg, b_reg):
    if "nc" not in _CACHE:
        _CACHE["nc"] = _build_program()
    nc = _CACHE["nc"]

    w_all = np.concatenate([w_off, w_mod], axis=0).astype(np.float32)
    wc = w_all.reshape(CO, 2, 128, KH, KW).transpose(1, 3, 4, 2, 0)
    wconv = np.ascontiguousarray(wc.reshape(2 * K * C, CO))
    # channel-outer PPIX order: [y-offsets, x-offsets, modulator]
    perm = [2 * k for k in range(K)] + [2 * k + 1 for k in range(K)] + list(range(2 * K, CO))
    wconv = np.ascontiguousarray(wconv[:, perm])
    # modulator = 2*sigmoid -> fold the 2x into the deform weights
    wd = (2.0 * np.asarray(w_reg, np.float32)).reshape(C, C, K).transpose(2, 1, 0)
    wdef = np.ascontiguousarray(wd.reshape(K * C, C))
    hk, wk = _host_maps(np.asarray(b_off, np.float32))

    bmk = np.broadcast_to(
        np.asarray(b_mod, np.float32)[None, :, None], (128, K, NF))
    shared = dict(
        wconv=wconv.astype(np.float16), wdef=wdef.astype(np.float16),
        breg=np.asarray(b_reg, np.float32)[:, None],
        bmod=np.ascontiguousarray(bmk),
        hkmap=hk, wkmap=wk, id27=np.eye(CO, dtype=np.float32),
    )
    in_maps = []
    for b in range(B):
        m = dict(shared)
        m["x"] = np.ascontiguousarray(np.asarray(x[b], np.float32).reshape(C, HW))
        m["ref"] = np.ascontiguousarray(np.asarray(ref_feats[b], np.float32).reshape(C, HW))
        in_maps.append(m)
    _CACHE["in_maps"] = in_maps

    res = run_bass_kernel_spmd(nc, in_maps, core_ids=list(range(B)))
    out = np.stack([np.asarray(res.results[b]["y"]).reshape(C, H, W) for b in range(B)])
    return out.astype(np.float32)
